# revision 26
# baseline (speedup 1.0000x reference)
"""Trainium2 Bass kernel for a diagonal-A linear dynamical system (LDS).

    Bu = inputs @ B            [B, T, S]
    h_t = h_{t-1} * A + Bu_t   (scan over T, diagonal A)
    y_t = h_t @ C              [B, T, O]

Shapes: inputs [16, 4096, 256], A [256], B [256, 256], C [256, 256],
h0 [256]; all float32.

Sharding: data-parallel over batch across 8 NeuronCores (2 batches per
core); A/B/C/h0 replicated.

v8: everything 2-byte on the wire. u/B/C/A/h0 are staged bf16 host-side
and y is returned bf16 (expanded to f32 on the host) -- host work is
free, HBM bytes halve. All input DMAs (one packed constant tensor + 16
u supertiles) go through the DMA xbar in transpose mode, back to back
on the sync queue; the y stores (copy mode) queue after them, so the
xbar never switches modes mid-stream (mode transitions serialize ALL
DMA rings). The DVE scan is the saturated engine in steady state; the
scan keeps fp32 internal state and Bu stays fp32 in PSUM. Measured rel
err ~4.8e-3 vs the 2e-2 gate.

Per-core dataflow (all tiles 128-partition):
  1. DMA-transpose Wall [1040, 128] bf16 -> [128, 1040] (B, C, A, h0).
  2. DMA-transpose u [TT t, 128 i] bf16 HBM -> uT [128 i, TT t] SBUF.
  3. PE matmul BuT[s, t] = B^T @ uT accumulated over i-halves into PSUM
     (fp32).
  4. DVE tensor_tensor_scan along t: state = A*state + Bu (fp32 internal
     state), chained across chunks via initial=prev last column. Output
     hT bf16 in SBUF.
  5. PE matmul y[t, o] = hT.T @ C (hT slices stationary, bf16 FWL).
  6. ACT copy y PSUM->SBUF bf16, DMA out bf16.
"""

import ml_dtypes
import numpy as np

import concourse.bacc as bacc
import concourse.bass as bass
import concourse.mybir as mybir
import concourse.tile as tile
from concourse import bass_utils

BATCH, T, D = 16, 4096, 256
NCORES = 8
BLOC = BATCH // NCORES  # batches per core
TT = 1024               # time supertile (DMA granularity)
NSUB = TT // 128        # 128-row subtiles per supertile
NJ = T // TT            # supertiles per sequence
SC = 512                # scan / PSUM chunk within a supertile
NTH = TT // SC          # chunks per supertile
F32 = mybir.dt.float32
BF16 = mybir.dt.bfloat16

# Packed constant layout (rows of the host-side [WROWS, 128] matrix;
# the xbar transposes it to a [128, WROWS] SBUF tile):
#   rows 0..2D-1    B^T  (k-th 128-row block of B, transposed)
#   rows 2D..4D-1   C^T
#   rows 4D..4D+1   A halves
#   rows 4D+2..4D+3 h0 halves
#   rows 4D+4..     pad to a multiple of 16
WROWS = 4 * D + 16

_CACHE: dict = {}


def _build_nc():
    nc = bacc.Bacc(trn_type="TRN2", target_bir_lowering=False)

    u = nc.dram_tensor("u", [BLOC, T, D], BF16, kind="ExternalInput")
    Walld = nc.dram_tensor("Wall", [WROWS, 128], BF16, kind="ExternalInput")
    y = nc.dram_tensor("y", [BLOC, T, D], BF16, kind="ExternalOutput")

    y_r = y[:].rearrange("b (j s p) o -> b j p s o", p=128, s=NSUB)

    mult = mybir.AluOpType.mult
    add = mybir.AluOpType.add

    bj = [(b, j) for b in range(BLOC) for j in range(NJ)]

    with tile.TileContext(nc) as tc:
        with (
            tc.tile_pool(name="const", bufs=1) as const,
            tc.tile_pool(name="upool", bufs=16) as upool,
            tc.tile_pool(name="sbuf", bufs=4) as sbuf,
            tc.tile_pool(name="hpool", bufs=1) as hpool,
            tc.tile_pool(name="ps_bu", bufs=4, space="PSUM") as ps_bu,
            tc.tile_pool(name="ps_y", bufs=4, space="PSUM") as ps_y,
        ):
            # --- all constants in ONE transpose-mode DMA on the scalar
            # HWDGE ring: it runs concurrently with the first u transpose
            # on the sync ring (same xbar mode, so no serialization).
            Wall = const.tile([128, WROWS], BF16, name="Wall")
            nc.scalar.dma_start_transpose(Wall, Walld[:])

            def B_slc(k, m):  # [128 i, 128 s] block of B^T
                return Wall[:, k * D + m * 128 : k * D + (m + 1) * 128]

            def C_slc(k):     # [128 s, D o] block of C^T
                return Wall[:, 2 * D + k * D : 2 * D + (k + 1) * D]

            # A/h0 as f32 (ACT scale APs must be FP32)
            Ah = const.tile([128, 4], F32, name="Ah")

            def A_col(m):
                return Ah[:, m : m + 1]

            def h0_col(m):
                return Ah[:, 2 + m : 3 + m]

            # --- input prefetch: the DMA xbar transposes [TT, 128] bf16
            # -> [128, TT] on the fly. Trigger cost is ~1.3us FIXED per
            # transpose DMA regardless of size, so use few, large ones.
            uT_tiles = {}
            for b, j in bj:
                for k in range(2):
                    uT = upool.tile([128, TT], BF16, tag="uT", name="uT")
                    nc.sync.dma_start_transpose(
                        uT,
                        u[b, j * TT : (j + 1) * TT, k * 128 : (k + 1) * 128],
                    )
                    uT_tiles[(b, j, k)] = uT

            # A_bc on DVE (idle until the first scan) so the first scan
            # isn't gated on the slower ACT pipeline.
            ones = const.tile([128, SC], F32, name="ones")
            nc.gpsimd.memset(ones, 1.0)
            nc.vector.tensor_copy(Ah, Wall[:, 4 * D : 4 * D + 4])
            A_bc = const.tile([128, 2, SC], F32, name="A_bc")
            for m in range(2):
                nc.vector.tensor_scalar_mul(A_bc[:, m], ones, A_col(m))

            # hidden states, [128s, b, mhalf, t]; persistent
            hT = hpool.tile([128, BLOC, 2, T], BF16, name="hT")

            for b, j in bj:
                uTs = [uT_tiles.pop((b, j, k)) for k in range(2)]

                for th in range(NTH):
                    t0 = j * TT + th * SC  # chunk start (abs time)
                    for m in range(2):
                        bu_ps = ps_bu.tile(
                            [128, SC], F32, tag="bu_ps", name="bu_ps"
                        )
                        for k in range(2):
                            nc.tensor.matmul(
                                bu_ps,
                                B_slc(k, m),
                                uTs[k][:, th * SC : (th + 1) * SC],
                                start=(k == 0),
                                stop=(k == 1),
                            )
                        init = (
                            h0_col(m)
                            if t0 == 0
                            else hT[:, b, m, t0 - 1 : t0]
                        )
                        nc.vector.tensor_tensor_scan(
                            hT[:, b, m, t0 : t0 + SC],
                            A_bc[:, m],
                            bu_ps,
                            init,
                            op0=mult,
                            op1=add,
                        )

                # y stores per supertile (triggers are ~fixed-cost, so
                # few and large), except the last supertile which stores
                # per half to shorten the final serial chain.
                last = (b, j) == bj[-1]
                y_sb = sbuf.tile([128, NSUB * D], BF16, tag="y_sb", name="y_sb")
                for half in range(NSUB // 2):
                    y_ps = ps_y.tile([128, 2 * D], F32, tag="y_ps", name="y_ps")
                    for i in range(2):
                        s_ = half * 2 + i
                        t0 = j * TT + s_ * 128
                        for k in range(2):
                            nc.tensor.matmul(
                                y_ps[:, i * D : (i + 1) * D],
                                hT[:, b, k, t0 : t0 + 128],
                                C_slc(k),
                                start=(k == 0),
                                stop=(k == 1),
                            )
                    # final half: copy on the (now idle) DVE and store via
                    # the scalar HWDGE ring, in parallel with the ACT copy
                    # + sync store of the previous half.
                    if last and half == NSUB // 2 - 1:
                        nc.vector.tensor_copy(
                            y_sb[:, half * 2 * D : (half + 1) * 2 * D], y_ps
                        )
                    else:
                        nc.scalar.copy(
                            y_sb[:, half * 2 * D : (half + 1) * 2 * D], y_ps
                        )
                    if last:
                        eng = nc.scalar if half == NSUB // 2 - 1 else nc.sync
                        eng.dma_start(
                            y_r[b, j][:, half * 2 : (half + 1) * 2],
                            y_sb[:, half * 2 * D : (half + 1) * 2 * D]
                            .rearrange("p (s o) -> p s o", s=2),
                        )
                if not last:
                    nc.sync.dma_start(
                        y_r[b, j], y_sb.rearrange("p (s o) -> p s o", s=NSUB)
                    )

    nc.compile()
    return nc


def _get_nc():
    if "nc" not in _CACHE:
        _CACHE["nc"] = _build_nc()
    return _CACHE["nc"]


def make_in_maps(inputs, A, B, C, h0):
    bf = ml_dtypes.bfloat16
    u = np.ascontiguousarray(np.asarray(inputs, dtype=np.float32).astype(bf))
    Bt = np.asarray(B, np.float32).reshape(2, 128, D).transpose(0, 2, 1)
    Ct = np.asarray(C, np.float32).reshape(2, 128, D).transpose(0, 2, 1)
    Wall = np.zeros((WROWS, 128), np.float32)
    Wall[0 : 2 * D] = Bt.reshape(2 * D, 128)
    Wall[2 * D : 4 * D] = Ct.reshape(2 * D, 128)
    Wall[4 * D : 4 * D + 2] = np.asarray(A, np.float32).reshape(2, 128)
    Wall[4 * D + 2 : 4 * D + 4] = np.asarray(h0, np.float32).reshape(2, 128)
    Wall = np.ascontiguousarray(Wall.astype(bf))
    return [
        {"u": np.ascontiguousarray(u[c * BLOC : (c + 1) * BLOC]), "Wall": Wall}
        for c in range(NCORES)
    ]


def kernel(inputs, A, B, C, h0, _trace=False):
    nc = _get_nc()
    in_maps = make_in_maps(inputs, A, B, C, h0)
    res = bass_utils.run_bass_kernel_spmd(
        nc, in_maps, core_ids=list(range(NCORES)), trace=_trace
    )
    out = np.concatenate([r["y"] for r in res.results], axis=0).astype(np.float32)
    if _trace:
        _CACHE["last_result"] = res
    return out


# revision 27
# speedup vs baseline: 1.1716x; 1.1716x over previous
"""Trainium2 Bass kernel for a diagonal-A linear dynamical system (LDS).

    Bu = inputs @ B            [B, T, S]
    h_t = h_{t-1} * A + Bu_t   (scan over T, diagonal A)
    y_t = h_t @ C              [B, T, O]

Shapes: inputs [16, 4096, 256], A [256], B [256, 256], C [256, 256],
h0 [256]; all float32.

Sharding: data-parallel over batch across 8 NeuronCores (2 batches per
core); A/B/C/h0 replicated.

v8: everything 2-byte on the wire. u/B/C/A/h0 are staged bf16 host-side
and y is returned bf16 (expanded to f32 on the host) -- host work is
free, HBM bytes halve. All input DMAs (one packed constant tensor + 16
u supertiles) go through the DMA xbar in transpose mode, back to back
on the sync queue; the y stores (copy mode) queue after them, so the
xbar never switches modes mid-stream (mode transitions serialize ALL
DMA rings). The DVE scan is the saturated engine in steady state; the
scan keeps fp32 internal state and Bu stays fp32 in PSUM. Measured rel
err ~4.8e-3 vs the 2e-2 gate.

Per-core dataflow (all tiles 128-partition):
  1. DMA-transpose Wall [1040, 128] bf16 -> [128, 1040] (B, C, A, h0).
  2. DMA-transpose u [TT t, 128 i] bf16 HBM -> uT [128 i, TT t] SBUF.
  3. PE matmul BuT[s, t] = B^T @ uT accumulated over i-halves into PSUM
     (fp32).
  4. DVE tensor_tensor_scan along t: state = A*state + Bu (fp32 internal
     state), chained across chunks via initial=prev last column. Output
     hT bf16 in SBUF.
  5. PE matmul y[t, o] = hT.T @ C (hT slices stationary, bf16 FWL).
  6. ACT copy y PSUM->SBUF bf16, DMA out bf16.
"""

import ml_dtypes
import numpy as np

import concourse.bacc as bacc
import concourse.bass as bass
import concourse.mybir as mybir
import concourse.tile as tile
from concourse import bass_utils

BATCH, T, D = 16, 4096, 256
NCORES = 8
BLOC = BATCH // NCORES  # batches per core
TT = 1024               # time supertile (DMA granularity)
NSUB = TT // 128        # 128-row subtiles per supertile
NJ = T // TT            # supertiles per sequence
SC = 512                # scan / PSUM chunk within a supertile
NTH = TT // SC          # chunks per supertile
F32 = mybir.dt.float32
BF16 = mybir.dt.bfloat16

# Packed constant layout (rows of the host-side [WROWS, 128] matrix;
# the xbar transposes it to a [128, WROWS] SBUF tile):
#   rows 0..2D-1    B^T  (k-th 128-row block of B, transposed)
#   rows 2D..4D-1   C^T
#   rows 4D..4D+1   A halves
#   rows 4D+2..4D+3 h0 halves
#   rows 4D+4..     pad to a multiple of 16
WROWS = 4 * D + 16

_CACHE: dict = {}


def _build_nc():
    nc = bacc.Bacc(trn_type="TRN2", target_bir_lowering=False)

    u = nc.dram_tensor("u", [BLOC, T, D], BF16, kind="ExternalInput")
    Walld = nc.dram_tensor("Wall", [WROWS, 128], BF16, kind="ExternalInput")
    y = nc.dram_tensor("y", [BLOC, T, D], BF16, kind="ExternalOutput")

    y_r = y[:].rearrange("b (j s p) o -> b j p s o", p=128, s=NSUB)

    mult = mybir.AluOpType.mult
    add = mybir.AluOpType.add

    bj = [(b, j) for b in range(BLOC) for j in range(NJ)]

    with tile.TileContext(nc) as tc:
        with (
            tc.tile_pool(name="const", bufs=1) as const,
            tc.tile_pool(name="upool", bufs=16) as upool,
            tc.tile_pool(name="sbuf", bufs=4) as sbuf,
            tc.tile_pool(name="hpool", bufs=1) as hpool,
            tc.tile_pool(name="ps_bu", bufs=4, space="PSUM") as ps_bu,
            tc.tile_pool(name="ps_y", bufs=4, space="PSUM") as ps_y,
        ):
            # --- all constants in ONE transpose-mode DMA on the scalar
            # HWDGE ring: it runs concurrently with the first u transpose
            # on the sync ring (same xbar mode, so no serialization).
            Wall = const.tile([128, WROWS], BF16, name="Wall")
            nc.scalar.dma_start_transpose(Wall, Walld[:])

            def B_slc(k, m):  # [128 i, 128 s] block of B^T
                return Wall[:, k * D + m * 128 : k * D + (m + 1) * 128]

            def C_slc(k):     # [128 s, D o] block of C^T
                return Wall[:, 2 * D + k * D : 2 * D + (k + 1) * D]

            # A/h0 as f32 (ACT scale APs must be FP32)
            Ah = const.tile([128, 4], F32, name="Ah")

            def A_col(m):
                return Ah[:, m : m + 1]

            def h0_col(m):
                return Ah[:, 2 + m : 3 + m]

            # --- input prefetch: the DMA xbar transposes [TT, 128] bf16
            # -> [128, TT] on the fly. Trigger cost is ~1.3us FIXED per
            # transpose DMA regardless of size, so use few, large ones.
            uT_tiles = {}
            for b, j in bj:
                for k in range(2):
                    uT = upool.tile([128, TT], BF16, tag="uT", name="uT")
                    nc.sync.dma_start_transpose(
                        uT,
                        u[b, j * TT : (j + 1) * TT, k * 128 : (k + 1) * 128],
                    )
                    uT_tiles[(b, j, k)] = uT

            # A_bc on DVE (idle until the first scan) so the first scan
            # isn't gated on the slower ACT pipeline.
            ones = const.tile([128, SC], F32, name="ones")
            nc.gpsimd.memset(ones, 1.0)
            nc.vector.tensor_copy(Ah, Wall[:, 4 * D : 4 * D + 4])
            A_bc = const.tile([128, 2, SC], F32, name="A_bc")
            for m in range(2):
                nc.vector.tensor_scalar_mul(A_bc[:, m], ones, A_col(m))

            # hidden states, [128s, b, mhalf, t]; persistent
            hT = hpool.tile([128, BLOC, 2, T], BF16, name="hT")

            for b, j in bj:
                uTs = [uT_tiles.pop((b, j, k)) for k in range(2)]

                for th in range(NTH):
                    t0 = j * TT + th * SC  # chunk start (abs time)
                    for m in range(2):
                        bu_ps = ps_bu.tile(
                            [128, SC], F32, tag="bu_ps", name="bu_ps"
                        )
                        for k in range(2):
                            nc.tensor.matmul(
                                bu_ps,
                                B_slc(k, m),
                                uTs[k][:, th * SC : (th + 1) * SC],
                                start=(k == 0),
                                stop=(k == 1),
                            )
                        init = (
                            h0_col(m)
                            if t0 == 0
                            else hT[:, b, m, t0 - 1 : t0]
                        )
                        nc.vector.tensor_tensor_scan(
                            hT[:, b, m, t0 : t0 + SC],
                            A_bc[:, m],
                            bu_ps,
                            init,
                            op0=mult,
                            op1=add,
                        )

                # y stores per supertile (triggers are ~fixed-cost, so
                # few and large), except the last supertile which stores
                # per half to shorten the final serial chain.
                last = (b, j) == bj[-1]
                y_sb = sbuf.tile([128, NSUB * D], BF16, tag="y_sb", name="y_sb")
                for half in range(NSUB // 2):
                    y_ps = ps_y.tile([128, 2 * D], F32, tag="y_ps", name="y_ps")
                    for i in range(2):
                        s_ = half * 2 + i
                        t0 = j * TT + s_ * 128
                        for k in range(2):
                            nc.tensor.matmul(
                                y_ps[:, i * D : (i + 1) * D],
                                hT[:, b, k, t0 : t0 + 128],
                                C_slc(k),
                                start=(k == 0),
                                stop=(k == 1),
                            )
                    nc.scalar.copy(
                        y_sb[:, half * 2 * D : (half + 1) * 2 * D], y_ps
                    )
                    if last:
                        nc.sync.dma_start(
                            y_r[b, j][:, half * 2 : (half + 1) * 2],
                            y_sb[:, half * 2 * D : (half + 1) * 2 * D]
                            .rearrange("p (s o) -> p s o", s=2),
                        )
                if not last:
                    nc.sync.dma_start(
                        y_r[b, j], y_sb.rearrange("p (s o) -> p s o", s=NSUB)
                    )

    nc.compile()
    return nc


def _get_nc():
    if "nc" not in _CACHE:
        _CACHE["nc"] = _build_nc()
    return _CACHE["nc"]


def make_in_maps(inputs, A, B, C, h0):
    bf = ml_dtypes.bfloat16
    u = np.ascontiguousarray(np.asarray(inputs, dtype=np.float32).astype(bf))
    Bt = np.asarray(B, np.float32).reshape(2, 128, D).transpose(0, 2, 1)
    Ct = np.asarray(C, np.float32).reshape(2, 128, D).transpose(0, 2, 1)
    Wall = np.zeros((WROWS, 128), np.float32)
    Wall[0 : 2 * D] = Bt.reshape(2 * D, 128)
    Wall[2 * D : 4 * D] = Ct.reshape(2 * D, 128)
    Wall[4 * D : 4 * D + 2] = np.asarray(A, np.float32).reshape(2, 128)
    Wall[4 * D + 2 : 4 * D + 4] = np.asarray(h0, np.float32).reshape(2, 128)
    Wall = np.ascontiguousarray(Wall.astype(bf))
    return [
        {"u": np.ascontiguousarray(u[c * BLOC : (c + 1) * BLOC]), "Wall": Wall}
        for c in range(NCORES)
    ]


def kernel(inputs, A, B, C, h0, _trace=False):
    nc = _get_nc()
    in_maps = make_in_maps(inputs, A, B, C, h0)
    res = bass_utils.run_bass_kernel_spmd(
        nc, in_maps, core_ids=list(range(NCORES)), trace=_trace
    )
    out = np.concatenate([r["y"] for r in res.results], axis=0).astype(np.float32)
    if _trace:
        _CACHE["last_result"] = res
    return out


# revision 28
# speedup vs baseline: 1.2173x; 1.0390x over previous
"""Trainium2 Bass kernel for a diagonal-A linear dynamical system (LDS).

    Bu = inputs @ B            [B, T, S]
    h_t = h_{t-1} * A + Bu_t   (scan over T, diagonal A)
    y_t = h_t @ C              [B, T, O]

Shapes: inputs [16, 4096, 256], A [256], B [256, 256], C [256, 256],
h0 [256]; all float32.

Sharding: data-parallel over batch across 8 NeuronCores (2 batches per
core); A/B/C/h0 replicated.

v8: everything 2-byte on the wire. u/B/C/A/h0 are staged bf16 host-side
and y is returned bf16 (expanded to f32 on the host) -- host work is
free, HBM bytes halve. All input DMAs (one packed constant tensor + 16
u supertiles) go through the DMA xbar in transpose mode, back to back
on the sync queue; the y stores (copy mode) queue after them, so the
xbar never switches modes mid-stream (mode transitions serialize ALL
DMA rings). The DVE scan is the saturated engine in steady state; the
scan keeps fp32 internal state and Bu stays fp32 in PSUM. Measured rel
err ~4.8e-3 vs the 2e-2 gate.

Per-core dataflow (all tiles 128-partition):
  1. DMA-transpose Wall [1040, 128] bf16 -> [128, 1040] (B, C, A, h0).
  2. DMA-transpose u [TT t, 128 i] bf16 HBM -> uT [128 i, TT t] SBUF.
  3. PE matmul BuT[s, t] = B^T @ uT accumulated over i-halves into PSUM
     (fp32).
  4. DVE tensor_tensor_scan along t: state = A*state + Bu (fp32 internal
     state), chained across chunks via initial=prev last column. Output
     hT bf16 in SBUF.
  5. PE matmul y[t, o] = hT.T @ C (hT slices stationary, bf16 FWL).
  6. ACT copy y PSUM->SBUF bf16, DMA out bf16.
"""

import ml_dtypes
import numpy as np

import concourse.bacc as bacc
import concourse.bass as bass
import concourse.mybir as mybir
import concourse.tile as tile
from concourse import bass_utils

BATCH, T, D = 16, 4096, 256
NCORES = 8
BLOC = BATCH // NCORES  # batches per core
TT = 1024               # time supertile (DMA granularity)
NSUB = TT // 128        # 128-row subtiles per supertile
NJ = T // TT            # supertiles per sequence
SC = 512                # scan / PSUM chunk within a supertile
NTH = TT // SC          # chunks per supertile
F32 = mybir.dt.float32
BF16 = mybir.dt.bfloat16

# Packed constant layout (rows of the host-side [WROWS, 128] matrix;
# the xbar transposes it to a [128, WROWS] SBUF tile):
#   rows 0..2D-1    B^T  (k-th 128-row block of B, transposed)
#   rows 2D..4D-1   C^T
#   rows 4D..4D+1   A halves
#   rows 4D+2..4D+3 h0 halves
#   rows 4D+4..     pad to a multiple of 16
WROWS = 4 * D + 16

_CACHE: dict = {}


def _build_nc():
    nc = bacc.Bacc(trn_type="TRN2", target_bir_lowering=False)

    u = nc.dram_tensor("u", [BLOC, T, D], BF16, kind="ExternalInput")
    Walld = nc.dram_tensor("Wall", [WROWS, 128], BF16, kind="ExternalInput")
    y = nc.dram_tensor("y", [BLOC, T, D], BF16, kind="ExternalOutput")

    y_r = y[:].rearrange("b (j s p) o -> b j p s o", p=128, s=NSUB)

    mult = mybir.AluOpType.mult
    add = mybir.AluOpType.add

    bj = [(b, j) for b in range(BLOC) for j in range(NJ)]

    with tile.TileContext(nc) as tc:
        with (
            tc.tile_pool(name="const", bufs=1) as const,
            tc.tile_pool(name="upool", bufs=16) as upool,
            tc.tile_pool(name="sbuf", bufs=4) as sbuf,
            tc.tile_pool(name="hpool", bufs=1) as hpool,
            tc.tile_pool(name="ps_bu", bufs=4, space="PSUM") as ps_bu,
            tc.tile_pool(name="ps_y", bufs=4, space="PSUM") as ps_y,
        ):
            # --- all constants in ONE transpose-mode DMA on the scalar
            # HWDGE ring: it runs concurrently with the first u transpose
            # on the sync ring (same xbar mode, so no serialization).
            Wall = const.tile([128, WROWS], BF16, name="Wall")
            nc.scalar.dma_start_transpose(Wall, Walld[:])

            def B_slc(k, m):  # [128 i, 128 s] block of B^T
                return Wall[:, k * D + m * 128 : k * D + (m + 1) * 128]

            def C_slc(k):     # [128 s, D o] block of C^T
                return Wall[:, 2 * D + k * D : 2 * D + (k + 1) * D]

            # A/h0 as f32 (ACT scale APs must be FP32)
            Ah = const.tile([128, 4], F32, name="Ah")

            def A_col(m):
                return Ah[:, m : m + 1]

            def h0_col(m):
                return Ah[:, 2 + m : 3 + m]

            # --- input prefetch: the DMA xbar transposes [TT, 128] bf16
            # -> [128, TT] on the fly. Trigger cost is ~1.3us FIXED per
            # transpose DMA regardless of size, so use few, large ones.
            uT_tiles = {}
            for b, j in bj:
                for k in range(2):
                    uT = upool.tile([128, TT], BF16, tag="uT", name="uT")
                    nc.sync.dma_start_transpose(
                        uT,
                        u[b, j * TT : (j + 1) * TT, k * 128 : (k + 1) * 128],
                    )
                    uT_tiles[(b, j, k)] = uT

            # A_bc on DVE (idle until the first scan) so the first scan
            # isn't gated on the slower ACT pipeline.
            ones = const.tile([128, SC], F32, name="ones")
            nc.gpsimd.memset(ones, 1.0)
            nc.vector.tensor_copy(Ah, Wall[:, 4 * D : 4 * D + 4])
            A_bc = const.tile([128, 2, SC], F32, name="A_bc")
            for m in range(2):
                nc.vector.tensor_scalar_mul(A_bc[:, m], ones, A_col(m))

            # hidden states, [128s, b, mhalf, t]; persistent
            hT = hpool.tile([128, BLOC, 2, T], BF16, name="hT")

            def emit_y(b, j):
                # y stores per supertile (triggers are ~fixed-cost, so
                # few and large), except the last supertile which stores
                # per half to shorten the final serial chain.
                last = (b, j) == bj[-1]
                y_sb = sbuf.tile([128, NSUB * D], BF16, tag="y_sb", name="y_sb")
                for half in range(NSUB // 2):
                    y_ps = ps_y.tile([128, 2 * D], F32, tag="y_ps", name="y_ps")
                    for i in range(2):
                        s_ = half * 2 + i
                        t0 = j * TT + s_ * 128
                        for k in range(2):
                            nc.tensor.matmul(
                                y_ps[:, i * D : (i + 1) * D],
                                hT[:, b, k, t0 : t0 + 128],
                                C_slc(k),
                                start=(k == 0),
                                stop=(k == 1),
                            )
                    nc.scalar.copy(
                        y_sb[:, half * 2 * D : (half + 1) * 2 * D], y_ps
                    )
                    if last:
                        nc.sync.dma_start(
                            y_r[b, j][:, half * 2 : (half + 1) * 2],
                            y_sb[:, half * 2 * D : (half + 1) * 2 * D]
                            .rearrange("p (s o) -> p s o", s=2),
                        )
                if not last:
                    nc.sync.dma_start(
                        y_r[b, j], y_sb.rearrange("p (s o) -> p s o", s=NSUB)
                    )

            # y work is emitted one supertile behind the Bu/scan work so
            # the scan supply chain (Bu matmuls) outranks y matmuls in
            # the scheduler's program-order priority.
            pend_y = None
            for b, j in bj:
                uTs = [uT_tiles.pop((b, j, k)) for k in range(2)]

                for th in range(NTH):
                    t0 = j * TT + th * SC  # chunk start (abs time)
                    for m in range(2):
                        bu_ps = ps_bu.tile(
                            [128, SC], F32, tag="bu_ps", name="bu_ps"
                        )
                        for k in range(2):
                            nc.tensor.matmul(
                                bu_ps,
                                B_slc(k, m),
                                uTs[k][:, th * SC : (th + 1) * SC],
                                start=(k == 0),
                                stop=(k == 1),
                            )
                        init = (
                            h0_col(m)
                            if t0 == 0
                            else hT[:, b, m, t0 - 1 : t0]
                        )
                        nc.vector.tensor_tensor_scan(
                            hT[:, b, m, t0 : t0 + SC],
                            A_bc[:, m],
                            bu_ps,
                            init,
                            op0=mult,
                            op1=add,
                        )

                if pend_y is not None:
                    emit_y(*pend_y)
                pend_y = (b, j)
            emit_y(*pend_y)

    nc.compile()
    return nc


def _get_nc():
    if "nc" not in _CACHE:
        _CACHE["nc"] = _build_nc()
    return _CACHE["nc"]


def make_in_maps(inputs, A, B, C, h0):
    bf = ml_dtypes.bfloat16
    u = np.ascontiguousarray(np.asarray(inputs, dtype=np.float32).astype(bf))
    Bt = np.asarray(B, np.float32).reshape(2, 128, D).transpose(0, 2, 1)
    Ct = np.asarray(C, np.float32).reshape(2, 128, D).transpose(0, 2, 1)
    Wall = np.zeros((WROWS, 128), np.float32)
    Wall[0 : 2 * D] = Bt.reshape(2 * D, 128)
    Wall[2 * D : 4 * D] = Ct.reshape(2 * D, 128)
    Wall[4 * D : 4 * D + 2] = np.asarray(A, np.float32).reshape(2, 128)
    Wall[4 * D + 2 : 4 * D + 4] = np.asarray(h0, np.float32).reshape(2, 128)
    Wall = np.ascontiguousarray(Wall.astype(bf))
    return [
        {"u": np.ascontiguousarray(u[c * BLOC : (c + 1) * BLOC]), "Wall": Wall}
        for c in range(NCORES)
    ]


def kernel(inputs, A, B, C, h0, _trace=False):
    nc = _get_nc()
    in_maps = make_in_maps(inputs, A, B, C, h0)
    res = bass_utils.run_bass_kernel_spmd(
        nc, in_maps, core_ids=list(range(NCORES)), trace=_trace
    )
    out = np.concatenate([r["y"] for r in res.results], axis=0).astype(np.float32)
    if _trace:
        _CACHE["last_result"] = res
    return out


# revision 30
# speedup vs baseline: 1.2237x; 1.0052x over previous
"""Trainium2 Bass kernel for a diagonal-A linear dynamical system (LDS).

    Bu = inputs @ B            [B, T, S]
    h_t = h_{t-1} * A + Bu_t   (scan over T, diagonal A)
    y_t = h_t @ C              [B, T, O]

Shapes: inputs [16, 4096, 256], A [256], B [256, 256], C [256, 256],
h0 [256]; all float32.

Sharding: data-parallel over batch across 8 NeuronCores (2 batches per
core); A/B/C/h0 replicated.

v8: everything 2-byte on the wire. u/B/C/A/h0 are staged bf16 host-side
and y is returned bf16 (expanded to f32 on the host) -- host work is
free, HBM bytes halve. All input DMAs (one packed constant tensor + 16
u supertiles) go through the DMA xbar in transpose mode, back to back
on the sync queue; the y stores (copy mode) queue after them, so the
xbar never switches modes mid-stream (mode transitions serialize ALL
DMA rings). The DVE scan is the saturated engine in steady state; the
scan keeps fp32 internal state and Bu stays fp32 in PSUM. Measured rel
err ~4.8e-3 vs the 2e-2 gate.

Per-core dataflow (all tiles 128-partition):
  1. DMA-transpose Wall [1040, 128] bf16 -> [128, 1040] (B, C, A, h0).
  2. DMA-transpose u [TT t, 128 i] bf16 HBM -> uT [128 i, TT t] SBUF.
  3. PE matmul BuT[s, t] = B^T @ uT accumulated over i-halves into PSUM
     (fp32).
  4. DVE tensor_tensor_scan along t: state = A*state + Bu (fp32 internal
     state), chained across chunks via initial=prev last column. Output
     hT bf16 in SBUF.
  5. PE matmul y[t, o] = hT.T @ C (hT slices stationary, bf16 FWL).
  6. ACT copy y PSUM->SBUF bf16, DMA out bf16.
"""

import ml_dtypes
import numpy as np

import concourse.bacc as bacc
import concourse.bass as bass
import concourse.mybir as mybir
import concourse.tile as tile
from concourse import bass_utils

BATCH, T, D = 16, 4096, 256
NCORES = 8
BLOC = BATCH // NCORES  # batches per core
TT = 1024               # time supertile (DMA granularity)
NSUB = TT // 128        # 128-row subtiles per supertile
NJ = T // TT            # supertiles per sequence
SC = 512                # scan / PSUM chunk within a supertile
NTH = TT // SC          # chunks per supertile
F32 = mybir.dt.float32
BF16 = mybir.dt.bfloat16

# Packed constant layout (rows of the host-side [WROWS, 128] matrix;
# the xbar transposes it to a [128, WROWS] SBUF tile):
#   rows 0..2D-1    B^T  (k-th 128-row block of B, transposed)
#   rows 2D..4D-1   C^T
#   rows 4D..4D+1   A halves
#   rows 4D+2..4D+3 h0 halves
#   rows 4D+4..     pad to a multiple of 16
WROWS = 4 * D + 16

_CACHE: dict = {}


def _build_nc():
    nc = bacc.Bacc(trn_type="TRN2", target_bir_lowering=False)

    u = nc.dram_tensor("u", [BLOC, T, D], BF16, kind="ExternalInput")
    Walld = nc.dram_tensor("Wall", [WROWS, 128], BF16, kind="ExternalInput")
    y = nc.dram_tensor("y", [BLOC, T, D], BF16, kind="ExternalOutput")

    y_r = y[:].rearrange("b (j s p) o -> b j p s o", p=128, s=NSUB)

    mult = mybir.AluOpType.mult
    add = mybir.AluOpType.add

    bj = [(b, j) for b in range(BLOC) for j in range(NJ)]

    with tile.TileContext(nc) as tc:
        with (
            tc.tile_pool(name="const", bufs=1) as const,
            tc.tile_pool(name="upool", bufs=16) as upool,
            tc.tile_pool(name="sbuf", bufs=4) as sbuf,
            tc.tile_pool(name="hpool", bufs=1) as hpool,
            tc.tile_pool(name="ps_bu", bufs=4, space="PSUM") as ps_bu,
            tc.tile_pool(name="ps_y", bufs=4, space="PSUM") as ps_y,
        ):
            # --- the scalar HWDGE ring carries the first supertile's k=1
            # transpose and then the packed constants, concurrent with the
            # sync ring's k=0 transpose (same xbar mode everywhere, so no
            # mode serialization). This shortens the ramp to the first
            # Bu matmul by ~1.4us.
            uT001 = upool.tile([128, TT], BF16, tag="uT", name="uT")
            nc.scalar.dma_start_transpose(uT001, u[0, 0:TT, 128:256])
            Wall = const.tile([128, WROWS], BF16, name="Wall")
            nc.scalar.dma_start_transpose(Wall, Walld[:])

            def B_slc(k, m):  # [128 i, 128 s] block of B^T
                return Wall[:, k * D + m * 128 : k * D + (m + 1) * 128]

            def C_slc(k):     # [128 s, D o] block of C^T
                return Wall[:, 2 * D + k * D : 2 * D + (k + 1) * D]

            # A/h0 as f32 (ACT scale APs must be FP32)
            Ah = const.tile([128, 4], F32, name="Ah")

            def A_col(m):
                return Ah[:, m : m + 1]

            def h0_col(m):
                return Ah[:, 2 + m : 3 + m]

            # --- input prefetch: the DMA xbar transposes [TT, 128] bf16
            # -> [128, TT] on the fly. Trigger cost is ~1.3us FIXED per
            # transpose DMA regardless of size, so use few, large ones.
            uT_tiles = {(0, 0, 1): uT001}
            for b, j in bj:
                for k in range(2):
                    if (b, j, k) in uT_tiles:
                        continue
                    uT = upool.tile([128, TT], BF16, tag="uT", name="uT")
                    nc.sync.dma_start_transpose(
                        uT,
                        u[b, j * TT : (j + 1) * TT, k * 128 : (k + 1) * 128],
                    )
                    uT_tiles[(b, j, k)] = uT

            # A_bc on DVE (idle until the first scan) so the first scan
            # isn't gated on the slower ACT pipeline.
            ones = const.tile([128, SC], F32, name="ones")
            nc.gpsimd.memset(ones, 1.0)
            nc.vector.tensor_copy(Ah, Wall[:, 4 * D : 4 * D + 4])
            A_bc = const.tile([128, 2, SC], F32, name="A_bc")
            for m in range(2):
                nc.vector.tensor_scalar_mul(A_bc[:, m], ones, A_col(m))

            # hidden states, [128s, b, mhalf, t]; persistent
            hT = hpool.tile([128, BLOC, 2, T], BF16, name="hT")

            def emit_y(b, j):
                # y stores per supertile (triggers are ~fixed-cost, so
                # few and large), except the last supertile which stores
                # per half to shorten the final serial chain.
                last = (b, j) == bj[-1]
                y_sb = sbuf.tile([128, NSUB * D], BF16, tag="y_sb", name="y_sb")
                for half in range(NSUB // 2):
                    y_ps = ps_y.tile([128, 2 * D], F32, tag="y_ps", name="y_ps")
                    for i in range(2):
                        s_ = half * 2 + i
                        t0 = j * TT + s_ * 128
                        for k in range(2):
                            nc.tensor.matmul(
                                y_ps[:, i * D : (i + 1) * D],
                                hT[:, b, k, t0 : t0 + 128],
                                C_slc(k),
                                start=(k == 0),
                                stop=(k == 1),
                            )
                    nc.scalar.copy(
                        y_sb[:, half * 2 * D : (half + 1) * 2 * D], y_ps
                    )
                    if last:
                        nc.sync.dma_start(
                            y_r[b, j][:, half * 2 : (half + 1) * 2],
                            y_sb[:, half * 2 * D : (half + 1) * 2 * D]
                            .rearrange("p (s o) -> p s o", s=2),
                        )
                if not last:
                    nc.sync.dma_start(
                        y_r[b, j], y_sb.rearrange("p (s o) -> p s o", s=NSUB)
                    )

            # y work is emitted one supertile behind the Bu/scan work so
            # the scan supply chain (Bu matmuls) outranks y matmuls in
            # the scheduler's program-order priority.
            pend_y = None
            for b, j in bj:
                uTs = [uT_tiles.pop((b, j, k)) for k in range(2)]

                for th in range(NTH):
                    t0 = j * TT + th * SC  # chunk start (abs time)
                    for m in range(2):
                        bu_ps = ps_bu.tile(
                            [128, SC], F32, tag="bu_ps", name="bu_ps"
                        )
                        for k in range(2):
                            nc.tensor.matmul(
                                bu_ps,
                                B_slc(k, m),
                                uTs[k][:, th * SC : (th + 1) * SC],
                                start=(k == 0),
                                stop=(k == 1),
                            )
                        init = (
                            h0_col(m)
                            if t0 == 0
                            else hT[:, b, m, t0 - 1 : t0]
                        )
                        nc.vector.tensor_tensor_scan(
                            hT[:, b, m, t0 : t0 + SC],
                            A_bc[:, m],
                            bu_ps,
                            init,
                            op0=mult,
                            op1=add,
                        )

                if pend_y is not None:
                    emit_y(*pend_y)
                pend_y = (b, j)
            emit_y(*pend_y)

    nc.compile()
    return nc


def _get_nc():
    if "nc" not in _CACHE:
        _CACHE["nc"] = _build_nc()
    return _CACHE["nc"]


def make_in_maps(inputs, A, B, C, h0):
    bf = ml_dtypes.bfloat16
    u = np.ascontiguousarray(np.asarray(inputs, dtype=np.float32).astype(bf))
    Bt = np.asarray(B, np.float32).reshape(2, 128, D).transpose(0, 2, 1)
    Ct = np.asarray(C, np.float32).reshape(2, 128, D).transpose(0, 2, 1)
    Wall = np.zeros((WROWS, 128), np.float32)
    Wall[0 : 2 * D] = Bt.reshape(2 * D, 128)
    Wall[2 * D : 4 * D] = Ct.reshape(2 * D, 128)
    Wall[4 * D : 4 * D + 2] = np.asarray(A, np.float32).reshape(2, 128)
    Wall[4 * D + 2 : 4 * D + 4] = np.asarray(h0, np.float32).reshape(2, 128)
    Wall = np.ascontiguousarray(Wall.astype(bf))
    return [
        {"u": np.ascontiguousarray(u[c * BLOC : (c + 1) * BLOC]), "Wall": Wall}
        for c in range(NCORES)
    ]


def kernel(inputs, A, B, C, h0, _trace=False):
    nc = _get_nc()
    in_maps = make_in_maps(inputs, A, B, C, h0)
    res = bass_utils.run_bass_kernel_spmd(
        nc, in_maps, core_ids=list(range(NCORES)), trace=_trace
    )
    out = np.concatenate([r["y"] for r in res.results], axis=0).astype(np.float32)
    if _trace:
        _CACHE["last_result"] = res
    return out
